# revision 1
# baseline (speedup 1.0000x reference)
"""Trainium2 Bass kernel for nn_GaussianMixture (mixture-of-5-Gaussians sampler).

Strategy: data-parallel over the row dim N=16384 across 8 NeuronCores
(2048 rows/core), MLP weights replicated. Per core, rows are processed in
two half-passes of 1024 rows to fit SBUF.

All matmuls run as float32r (TF32) on the PE at 1 cycle/row. Activations
are kept feature-major ([h_features, n_rows]) through the hidden layers so
no activation transposes are needed; the final expert layer flips
orientation (uses the feature-major hidden state as lhsT) to produce
row-major output directly, so noise / sampling / weighted-combine all run
row-major with per-partition scalar weights. Only the input c is
transposed (once, on the PE).
"""
import sys

sys.path.insert(0, "/opt/trn_rl_repo")

from contextlib import ExitStack

import numpy as np

import concourse.bass as bass
import concourse.tile as tile
from concourse import bacc, mybir
from concourse.bass_utils import run_bass_kernel_spmd
from concourse.masks import make_identity

F32 = mybir.dt.float32
F32R = mybir.dt.float32r
AF = mybir.ActivationFunctionType
ALU = mybir.AluOpType
AX = mybir.AxisListType

N_CORES = 8
N, CDIM, FDIM, HDIM, K = 16384, 512, 512, 1024, 5
F2 = 2 * FDIM
WEIGHT = 5.0
EPS = 1e-20

CT = CDIM // 128  # 4 c-feature tiles
HT = HDIM // 128  # 8 h-feature tiles


def build_program(nl: int):
    """Build the per-core program for nl rows (nl=2048 for the real run)."""
    assert nl % 256 == 0
    nh = nl // 2          # rows per half-pass
    rt = nh // 128        # row-tiles per half
    nb = min(512, nh)     # n-block (matmul moving size)
    nbc = nh // nb        # n-blocks per half
    ntl = nb // 128       # row-tiles per n-block

    nc = bacc.Bacc("TRN2", target_bir_lowering=False, debug=False)

    c_d = nc.dram_tensor("c", [nl, CDIM], F32, kind="ExternalInput").ap()
    noise_d = nc.dram_tensor("noise", [K, nl, FDIM], F32, kind="ExternalInput").ap()
    gu_d = nc.dram_tensor("gumbel_u", [nl, K], F32, kind="ExternalInput").ap()
    pw0_d = nc.dram_tensor("pw0", [CDIM, HDIM], F32, kind="ExternalInput").ap()
    pb0_d = nc.dram_tensor("pb0", [HDIM], F32, kind="ExternalInput").ap()
    pw1_d = nc.dram_tensor("pw1", [HDIM, HDIM], F32, kind="ExternalInput").ap()
    pb1_d = nc.dram_tensor("pb1", [HDIM], F32, kind="ExternalInput").ap()
    pw2_d = nc.dram_tensor("pw2", [HDIM, K], F32, kind="ExternalInput").ap()
    pb2_d = nc.dram_tensor("pb2", [K], F32, kind="ExternalInput").ap()
    gw0_d = nc.dram_tensor("gw0", [K, CDIM, HDIM], F32R, kind="ExternalInput").ap()
    gb0_d = nc.dram_tensor("gb0", [K, HDIM], F32, kind="ExternalInput").ap()
    gw1_d = nc.dram_tensor("gw1", [K, HDIM, HDIM], F32R, kind="ExternalInput").ap()
    gb1_d = nc.dram_tensor("gb1", [K, HDIM], F32, kind="ExternalInput").ap()
    gw2_d = nc.dram_tensor("gw2", [K, HDIM, F2], F32R, kind="ExternalInput").ap()
    gb2_d = nc.dram_tensor("gb2", [K, F2], F32, kind="ExternalInput").ap()
    out_d = nc.dram_tensor("out", [nl, FDIM], F32, kind="ExternalOutput").ap()

    with tile.TileContext(nc) as tc:
        with ExitStack() as gctx:
            const = gctx.enter_context(tc.tile_pool(name="const", bufs=1))
            ps_mm = gctx.enter_context(
                tc.tile_pool(name="ps_mm", bufs=4, space="PSUM")
            )
            ps_l3 = gctx.enter_context(
                tc.tile_pool(name="ps_l3", bufs=2, space="PSUM")
            )

            # one packed const tile: identity | pb2 broadcast | eps
            constt = const.tile([128, 134], F32, tag="constt")
            ident = constt[:, 0:128]
            pb2_b = constt[:, 128:133]
            eps_b = constt[:, 133:134]
            make_identity(nc, ident)
            nc.gpsimd.dma_start(out=pb2_b, in_=pb2_d.partition_broadcast(128))
            nc.vector.memset(eps_b, EPS)

            for half in range(2):
                r0 = half * nh
                with ExitStack() as hctx:
                    sb = hctx.enter_context(
                        tc.tile_pool(name=f"sb{half}", bufs=1)
                    )
                    act = hctx.enter_context(
                        tc.tile_pool(name=f"act{half}", bufs=1)
                    )
                    tmp = hctx.enter_context(
                        tc.tile_pool(name=f"tmp{half}", bufs=2)
                    )
                    nzp = hctx.enter_context(
                        tc.tile_pool(name=f"nz{half}", bufs=2)
                    )
                    pre = hctx.enter_context(
                        tc.tile_pool(name=f"pre{half}", bufs=1)
                    )

                    cT = sb.tile([128, CT, nh], F32R, tag="cT")

                    # packed per-row small arrays: logits | gu | lg1 | sc | wgt
                    smalls = sb.tile([128, 5, rt, K], F32, tag="smalls")
                    logits = smalls[:, 0]
                    gu = smalls[:, 1]
                    lg1 = smalls[:, 2]
                    sc = smalls[:, 3]
                    wgt = smalls[:, 4]
                    acc = sb.tile([128, rt, FDIM], F32, tag="acc")

                    # prefetch expert 0's first-layer weights during probs
                    gw0_first = pre.tile([128, CT, HDIM], F32R, tag="gw0")
                    nc.sync.dma_start(
                        out=gw0_first,
                        in_=gw0_d[0].rearrange("(t p) h -> p t h", p=128),
                    )

                    # ---- probs MLP ----
                    with ExitStack() as pctx:
                        pw = pctx.enter_context(
                            tc.tile_pool(name=f"pw{half}", bufs=1)
                        )
                        # feature-major c via PE transpose: cT32 (exact, for
                        # the fp32 probs MLP, freed after probs) and cT
                        # (f32r-rounded, for the TF32 expert MLPs).
                        cT32 = pw.tile([128, CT, nh], F32, tag="cT32")
                        for r in range(rt):
                            c_row = tmp.tile([128, CDIM], F32, tag="c_row", bufs=2)
                            nc.sync.dma_start(
                                out=c_row,
                                in_=c_d[r0 + r * 128 : r0 + (r + 1) * 128, :],
                            )
                            for ct in range(CT):
                                pst = ps_mm.tile([128, 128], F32, tag="mm")
                                nc.tensor.transpose(
                                    pst[:], c_row[:, ct * 128 : (ct + 1) * 128], ident
                                )
                                nc.vector.tensor_copy(
                                    cT[:, ct, r * 128 : (r + 1) * 128], pst[:]
                                )
                                nc.scalar.copy(
                                    cT32[:, ct, r * 128 : (r + 1) * 128], pst[:]
                                )
                        pw0_s = pw.tile([128, CT, HDIM], F32, tag="w0")
                        nc.sync.dma_start(
                            out=pw0_s, in_=pw0_d.rearrange("(t p) h -> p t h", p=128)
                        )
                        pw1_s = pw.tile([128, HT, HDIM], F32, tag="w1")
                        nc.sync.dma_start(
                            out=pw1_s, in_=pw1_d.rearrange("(t p) h -> p t h", p=128)
                        )
                        pw2_s = pw.tile([128, HT, K], F32, tag="w2")
                        nc.gpsimd.dma_start(
                            out=pw2_s, in_=pw2_d.rearrange("(t p) k -> p t k", p=128)
                        )
                        pbb = pw.tile([128, 2, HT], F32, tag="pbb")
                        nc.gpsimd.dma_start(
                            out=pbb[:, 0, :],
                            in_=pb0_d.rearrange("(t p) -> p t", p=128),
                        )
                        nc.gpsimd.dma_start(
                            out=pbb[:, 1, :],
                            in_=pb1_d.rearrange("(t p) -> p t", p=128),
                        )

                        for b in range(nbc):
                            cs = slice(b * nb, (b + 1) * nb)
                            h0 = act.tile([128, HT, nb], F32, tag="a0")
                            for ht in range(HT):
                                ps = ps_mm.tile([128, nb], F32, tag="mm")
                                for ct in range(CT):
                                    nc.tensor.matmul(
                                        ps[:],
                                        pw0_s[:, ct, ht * 128 : (ht + 1) * 128],
                                        cT32[:, ct, cs],
                                        start=(ct == 0),
                                        stop=(ct == CT - 1),
                                    )
                                nc.scalar.activation(
                                    h0[:, ht, :], ps[:], AF.Relu,
                                    bias=pbb[:, 0, ht : ht + 1],
                                )
                            h1 = act.tile([128, HT, nb], F32, tag="a1")
                            for h2 in range(HT):
                                ps = ps_mm.tile([128, nb], F32, tag="mm")
                                for h_1 in range(HT):
                                    nc.tensor.matmul(
                                        ps[:],
                                        pw1_s[:, h_1, h2 * 128 : (h2 + 1) * 128],
                                        h0[:, h_1, :],
                                        start=(h_1 == 0),
                                        stop=(h_1 == HT - 1),
                                    )
                                nc.scalar.activation(
                                    h1[:, h2, :], ps[:], AF.Relu,
                                    bias=pbb[:, 1, h2 : h2 + 1],
                                )
                            for t in range(ntl):
                                r = b * ntl + t
                                psl = ps_mm.tile([128, K], F32, tag="mm")
                                for ht in range(HT):
                                    nc.tensor.matmul(
                                        psl[:],
                                        h1[:, ht, t * 128 : (t + 1) * 128],
                                        pw2_s[:, ht, :],
                                        start=(ht == 0),
                                        stop=(ht == HT - 1),
                                    )
                                nc.vector.tensor_add(
                                    logits[:, r, :], psl[:], pb2_b
                                )

                    # ---- gumbel-max + softmax weights ----
                    nc.gpsimd.dma_start(
                        out=gu,
                        in_=gu_d[r0 : r0 + nh, :].rearrange(
                            "(t p) k -> p t k", p=128
                        ),
                    )
                    # lg1 = log(u + EPS); then lg1 <- log(-lg1 + EPS) = -gumbel
                    nc.scalar.activation(lg1, gu, AF.Ln, bias=eps_b)
                    nc.scalar.activation(lg1, lg1, AF.Ln, bias=eps_b, scale=-1.0)
                    # sc = logits + gumbel
                    nc.vector.tensor_sub(sc, logits, lg1)
                    for r in range(rt):
                        # packed per-r temps: m1|mx|nmx|sm|rs | oh5 | ex | ps_t
                        tg = tmp.tile([128, 20], F32, tag="tg")
                        m1 = tg[:, 0:1]
                        mx = tg[:, 1:2]
                        nmx = tg[:, 2:3]
                        sm = tg[:, 3:4]
                        rs = tg[:, 4:5]
                        oh5 = tg[:, 5:10]
                        ex = tg[:, 10:15]
                        ps_t = tg[:, 15:20]
                        nc.vector.tensor_reduce(
                            m1, sc[:, r, :], axis=AX.X, op=ALU.max
                        )
                        nc.vector.tensor_scalar(
                            oh5, sc[:, r, :], m1, WEIGHT, ALU.is_ge, ALU.mult
                        )
                        nc.vector.tensor_reduce(
                            mx, logits[:, r, :], axis=AX.X, op=ALU.max
                        )
                        nc.vector.tensor_scalar_mul(nmx, mx, -1.0)
                        nc.scalar.activation(
                            ex, logits[:, r, :], AF.Exp, bias=nmx
                        )
                        nc.vector.tensor_reduce(sm, ex, axis=AX.X, op=ALU.add)
                        nc.vector.reciprocal(rs, sm)
                        nc.vector.tensor_scalar_mul(ps_t, ex, rs)
                        nc.vector.tensor_add(ps_t, ps_t, oh5)
                        nc.vector.tensor_scalar_mul(
                            wgt[:, r, :], ps_t, 1.0 / (1.0 + WEIGHT)
                        )

                    # ---- experts ----
                    with ExitStack() as ectx:
                        ew = ectx.enter_context(
                            tc.tile_pool(name=f"ew{half}", bufs=1)
                        )
                        for k in range(K):
                            if k == 0:
                                gw0_s = gw0_first
                            else:
                                gw0_s = pre.tile([128, CT, HDIM], F32R, tag="gw0")
                                nc.sync.dma_start(
                                    out=gw0_s,
                                    in_=gw0_d[k].rearrange(
                                        "(t p) h -> p t h", p=128
                                    ),
                                )
                            gw1_s = ew.tile([128, HT, HDIM], F32R, tag="gw1")
                            nc.sync.dma_start(
                                out=gw1_s,
                                in_=gw1_d[k].rearrange("(t p) h -> p t h", p=128),
                            )
                            gw2_s = ew.tile([128, HT, F2], F32R, tag="gw2")
                            nc.sync.dma_start(
                                out=gw2_s,
                                in_=gw2_d[k].rearrange("(t p) f -> p t f", p=128),
                            )
                            gbb = ew.tile([128, 2, HT], F32, tag="gbb")
                            nc.gpsimd.dma_start(
                                out=gbb[:, 0, :],
                                in_=gb0_d[k].rearrange("(t p) -> p t", p=128),
                            )
                            nc.gpsimd.dma_start(
                                out=gbb[:, 1, :],
                                in_=gb1_d[k].rearrange("(t p) -> p t", p=128),
                            )
                            bb = ew.tile([128, 2, FDIM], F32, tag="bb")
                            nc.gpsimd.dma_start(
                                out=bb[:, 0, :],
                                in_=gb2_d[k, 0:FDIM].partition_broadcast(128),
                            )
                            nc.gpsimd.dma_start(
                                out=bb[:, 1, :],
                                in_=gb2_d[k, FDIM:F2].partition_broadcast(128),
                            )

                            for b in range(nbc):
                                cs = slice(b * nb, (b + 1) * nb)
                                g0 = act.tile([128, HT, nb], F32R, tag="a0")
                                for ht in range(HT):
                                    ps = ps_mm.tile([128, nb], F32, tag="mm")
                                    for ct in range(CT):
                                        nc.tensor.matmul(
                                            ps[:],
                                            gw0_s[:, ct, ht * 128 : (ht + 1) * 128],
                                            cT[:, ct, cs],
                                            start=(ct == 0),
                                            stop=(ct == CT - 1),
                                        )
                                    nc.scalar.activation(
                                        g0[:, ht, :], ps[:], AF.Relu,
                                        bias=gbb[:, 0, ht : ht + 1],
                                    )
                                g1 = act.tile([128, HT, nb], F32R, tag="a1")
                                for h2 in range(HT):
                                    ps = ps_mm.tile([128, nb], F32, tag="mm")
                                    for h_1 in range(HT):
                                        nc.tensor.matmul(
                                            ps[:],
                                            gw1_s[:, h_1, h2 * 128 : (h2 + 1) * 128],
                                            g0[:, h_1, :],
                                            start=(h_1 == 0),
                                            stop=(h_1 == HT - 1),
                                        )
                                    nc.scalar.activation(
                                        g1[:, h2, :], ps[:], AF.Relu,
                                        bias=gbb[:, 1, h2 : h2 + 1],
                                    )
                                # layer 3: row-major output [n, 2F]
                                for t in range(ntl):
                                    r = b * ntl + t
                                    ts_ = slice(t * 128, (t + 1) * 128)
                                    ps_m = ps_l3.tile([128, FDIM], F32, tag="m")
                                    ps_lv = ps_l3.tile([128, FDIM], F32, tag="lv")
                                    for ht in range(HT):
                                        nc.tensor.matmul(
                                            ps_m[:],
                                            g1[:, ht, ts_],
                                            gw2_s[:, ht, 0:FDIM],
                                            start=(ht == 0),
                                            stop=(ht == HT - 1),
                                        )
                                    for ht in range(HT):
                                        nc.tensor.matmul(
                                            ps_lv[:],
                                            g1[:, ht, ts_],
                                            gw2_s[:, ht, FDIM:F2],
                                            start=(ht == 0),
                                            stop=(ht == HT - 1),
                                        )
                                    o_m = tmp.tile([128, FDIM], F32, tag="o_m")
                                    nc.vector.tensor_add(
                                        o_m[:], ps_m[:], bb[:, 0, :]
                                    )
                                    o_lv = tmp.tile(
                                        [128, FDIM], F32, tag="o_lv", bufs=1
                                    )
                                    nc.vector.tensor_add(
                                        o_lv[:], ps_lv[:], bb[:, 1, :]
                                    )
                                    std = tmp.tile([128, FDIM], F32, tag="std")
                                    nc.scalar.activation(
                                        std[:], o_lv[:], AF.Exp, scale=0.5
                                    )
                                    nz_t = nzp.tile([128, FDIM], F32, tag="nz")
                                    nc.sync.dma_start(
                                        out=nz_t,
                                        in_=noise_d[
                                            k, r0 + r * 128 : r0 + (r + 1) * 128, :
                                        ],
                                    )
                                    smp = tmp.tile([128, FDIM], F32, tag="smp")
                                    nc.vector.tensor_mul(smp[:], nz_t[:], std[:])
                                    nc.vector.tensor_add(smp[:], smp[:], o_m[:])
                                    wv = wgt[:, r, k : k + 1]
                                    if k == 0:
                                        nc.vector.tensor_scalar_mul(
                                            acc[:, r, :], smp[:], wv
                                        )
                                    else:
                                        nc.vector.tensor_scalar_mul(
                                            smp[:], smp[:], wv
                                        )
                                        nc.vector.tensor_add(
                                            acc[:, r, :], acc[:, r, :], smp[:]
                                        )

                    for r in range(rt):
                        nc.sync.dma_start(
                            out=out_d[r0 + r * 128 : r0 + (r + 1) * 128, :],
                            in_=acc[:, r, :],
                        )
    nc.compile()
    return nc


_PROGRAM_CACHE = {}


def get_program(nl: int):
    if nl not in _PROGRAM_CACHE:
        _PROGRAM_CACHE[nl] = build_program(nl)
    return _PROGRAM_CACHE[nl]


def make_in_maps(inputs: dict, n_cores: int = N_CORES):
    nl = inputs["c"].shape[0] // n_cores
    shared = {}
    for name in ("pw0", "pb0", "pw1", "pb1", "pw2", "pb2",
                 "gw0", "gb0", "gw1", "gb1", "gw2", "gb2"):
        shared[name] = np.ascontiguousarray(
            np.asarray(inputs[name], dtype=np.float32)
        )
    c = np.asarray(inputs["c"], dtype=np.float32)
    noise = np.asarray(inputs["noise"], dtype=np.float32)
    gu = np.asarray(inputs["gumbel_u"], dtype=np.float32)
    in_maps = []
    for i in range(n_cores):
        rows = slice(i * nl, (i + 1) * nl)
        m = dict(shared)
        m["c"] = np.ascontiguousarray(c[rows])
        m["noise"] = np.ascontiguousarray(noise[:, rows, :])
        m["gumbel_u"] = np.ascontiguousarray(gu[rows])
        in_maps.append(m)
    return in_maps


def kernel(**inputs) -> np.ndarray:
    nc = get_program(N // N_CORES)
    in_maps = make_in_maps(inputs)
    res = run_bass_kernel_spmd(nc, in_maps, core_ids=list(range(N_CORES)))
    return np.concatenate(
        [res.results[i]["out"] for i in range(N_CORES)], axis=0
    )



# revision 7
# speedup vs baseline: 1.0738x; 1.0738x over previous
"""Trainium2 Bass kernel for nn_GaussianMixture (mixture-of-5-Gaussians sampler).

Strategy: data-parallel over the row dim N=16384 across 8 NeuronCores
(2048 rows/core), MLP weights replicated.

v2 vs baseline:
- Single 2048-row pass (no half-passes): the fp32 probs phase runs once,
  so the PE-clock throttle it triggers is paid once.
- Expert loop outermost: each expert's weights are DMA'd once per core
  (not once per half), double-buffered so expert k+1's weights stream in
  during expert k's compute.
- Expert matmuls in bf16 (weights converted host-side): ~5% faster issue
  rate than f32r and half the LDWEIGHTS/DMA bytes. Probs MLP stays fp32
  (the Gumbel argmax index is flip-sensitive to logit error).
- Activations stay feature-major ([h_features, n_rows]) through hidden
  layers; the final expert layer uses the hidden state as lhsT to emit
  row-major output, so sampling/weighted-combine run row-major.
"""
import sys

sys.path.insert(0, "/opt/trn_rl_repo")

from contextlib import ExitStack

import numpy as np

import concourse.bass as bass
import concourse.tile as tile
from concourse import bacc, mybir
from concourse.bass_utils import run_bass_kernel_spmd
from concourse.masks import make_identity

F32 = mybir.dt.float32
BF16 = mybir.dt.bfloat16
AF = mybir.ActivationFunctionType
ALU = mybir.AluOpType
AX = mybir.AxisListType

N_CORES = 8
N, CDIM, FDIM, HDIM, K = 16384, 512, 512, 1024, 5
F2 = 2 * FDIM
WEIGHT = 5.0
EPS = 1e-20

CT = CDIM // 128  # 4 c-feature tiles
HT = HDIM // 128  # 8 h-feature tiles


def build_program(nl: int):
    """Build the per-core program for nl rows (nl=2048 for the real run)."""
    nb = min(512, nl)     # n-block (matmul moving size)
    nbc = nl // nb        # n-blocks
    ntl = nb // 128       # row-tiles per n-block
    rt = nl // 128        # row-tiles total

    nc = bacc.Bacc("TRN2", target_bir_lowering=False, debug=False)

    c_d = nc.dram_tensor("c", [nl, CDIM], F32, kind="ExternalInput").ap()
    noise_d = nc.dram_tensor("noise", [K, nl, FDIM], F32, kind="ExternalInput").ap()
    gu_d = nc.dram_tensor("gumbel_u", [nl, K], F32, kind="ExternalInput").ap()
    pw0_d = nc.dram_tensor("pw0", [CDIM, HDIM], F32, kind="ExternalInput").ap()
    pb0_d = nc.dram_tensor("pb0", [HDIM], F32, kind="ExternalInput").ap()
    pw1_d = nc.dram_tensor("pw1", [HDIM, HDIM], F32, kind="ExternalInput").ap()
    pb1_d = nc.dram_tensor("pb1", [HDIM], F32, kind="ExternalInput").ap()
    pw2_d = nc.dram_tensor("pw2", [HDIM, K], F32, kind="ExternalInput").ap()
    pb2_d = nc.dram_tensor("pb2", [K], F32, kind="ExternalInput").ap()
    gw0_d = nc.dram_tensor("gw0", [K, CDIM, HDIM], BF16, kind="ExternalInput").ap()
    gb0_d = nc.dram_tensor("gb0", [K, HDIM], F32, kind="ExternalInput").ap()
    gw1_d = nc.dram_tensor("gw1", [K, HDIM, HDIM], BF16, kind="ExternalInput").ap()
    gb1_d = nc.dram_tensor("gb1", [K, HDIM], F32, kind="ExternalInput").ap()
    gw2_d = nc.dram_tensor("gw2", [K, HDIM, F2], BF16, kind="ExternalInput").ap()
    gb2_d = nc.dram_tensor("gb2", [K, F2], F32, kind="ExternalInput").ap()
    out_d = nc.dram_tensor("out", [nl, FDIM], F32, kind="ExternalOutput").ap()

    with tile.TileContext(nc) as tc:
        with ExitStack() as gctx:
            const = gctx.enter_context(tc.tile_pool(name="const", bufs=1))
            ps_mm = gctx.enter_context(
                tc.tile_pool(name="ps_mm", bufs=4, space="PSUM")
            )
            ps_l3 = gctx.enter_context(
                tc.tile_pool(name="ps_l3", bufs=2, space="PSUM")
            )
            sb = gctx.enter_context(tc.tile_pool(name="sb", bufs=1))
            ew = gctx.enter_context(tc.tile_pool(name="ew", bufs=2))

            # one packed const tile: identity | pb2 broadcast | eps
            constt = const.tile([128, 134], F32, tag="constt")
            ident = constt[:, 0:128]
            pb2_b = constt[:, 128:133]
            eps_b = constt[:, 133:134]
            make_identity(nc, ident)
            nc.gpsimd.dma_start(out=pb2_b, in_=pb2_d.partition_broadcast(128))
            nc.vector.memset(eps_b, EPS)

            # bf16 feature-major c for the expert MLPs, per block
            cT_bf = [
                sb.tile([128, CT, nb], BF16, tag=f"cTb{b}", name=f"cTb{b}")
                for b in range(nbc)
            ]
            # packed per-row small arrays: logits | gu | lg1 | sc | wgt
            smalls = sb.tile([128, 5, rt, K], F32, tag="smalls")
            logits = smalls[:, 0]
            gu = smalls[:, 1]
            lg1 = smalls[:, 2]
            sc = smalls[:, 3]
            wgt = smalls[:, 4]

            def load_expert(k):
                """Allocate + start DMA for expert k's weights (bf16) and
                biases. gw0/gw1/biases double-buffer; gw2 is single-buffered
                (only needed at L3) and issued LAST on the gpsimd queue so
                its WAR wait on expert k-1's L3 reads doesn't delay the
                bias transfers."""
                gw0_s = ew.tile([128, CT, HDIM], BF16, tag="gw0")
                nc.sync.dma_start(
                    out=gw0_s, in_=gw0_d[k].rearrange("(t p) h -> p t h", p=128)
                )
                gw1_s = ew.tile([128, HT, HDIM], BF16, tag="gw1")
                nc.sync.dma_start(
                    out=gw1_s, in_=gw1_d[k].rearrange("(t p) h -> p t h", p=128)
                )
                gbb = ew.tile([128, 2, HT], F32, tag="gbb")
                nc.gpsimd.dma_start(
                    out=gbb[:, 0, :], in_=gb0_d[k].rearrange("(t p) -> p t", p=128)
                )
                nc.gpsimd.dma_start(
                    out=gbb[:, 1, :], in_=gb1_d[k].rearrange("(t p) -> p t", p=128)
                )
                bb = ew.tile([128, 2, FDIM], F32, tag="bb")
                nc.gpsimd.dma_start(
                    out=bb[:, 0, :], in_=gb2_d[k, 0:FDIM].partition_broadcast(128)
                )
                nc.gpsimd.dma_start(
                    out=bb[:, 1, :], in_=gb2_d[k, FDIM:F2].partition_broadcast(128)
                )
                gw2_s = ew.tile([128, HT, F2], BF16, tag="gw2", bufs=1)
                nc.gpsimd.dma_start(
                    out=gw2_s, in_=gw2_d[k].rearrange("(t p) f -> p t f", p=128)
                )
                return gw0_s, gw1_s, gw2_s, gbb, bb

            # prefetch expert 0's weights right away, behind nothing
            ew_next = load_expert(0)

            # ---- probs MLP (fp32) ----
            with ExitStack() as pctx:
                pw = pctx.enter_context(tc.tile_pool(name="pw", bufs=1))
                act = pctx.enter_context(tc.tile_pool(name="act", bufs=1))
                tmp = pctx.enter_context(tc.tile_pool(name="tmp", bufs=2))

                pw0_s = pw.tile([128, CT, HDIM], F32, tag="w0")
                nc.sync.dma_start(
                    out=pw0_s, in_=pw0_d.rearrange("(t p) h -> p t h", p=128)
                )
                pw1_s = pw.tile([128, HT, HDIM], F32, tag="w1")
                nc.sync.dma_start(
                    out=pw1_s, in_=pw1_d.rearrange("(t p) h -> p t h", p=128)
                )
                pw2_s = pw.tile([128, HT, K], F32, tag="w2")
                nc.gpsimd.dma_start(
                    out=pw2_s, in_=pw2_d.rearrange("(t p) k -> p t k", p=128)
                )
                pbb = pw.tile([128, 2, HT], F32, tag="pbb")
                nc.gpsimd.dma_start(
                    out=pbb[:, 0, :], in_=pb0_d.rearrange("(t p) -> p t", p=128)
                )
                nc.gpsimd.dma_start(
                    out=pbb[:, 1, :], in_=pb1_d.rearrange("(t p) -> p t", p=128)
                )
                nc.gpsimd.dma_start(
                    out=gu, in_=gu_d.rearrange("(t p) k -> p t k", p=128)
                )

                for b in range(nbc):
                    # fp32 feature-major c for this block (rotating buffer)
                    cT32 = pw.tile([128, CT, nb], F32, tag="cT", bufs=2)
                    for t in range(ntl):
                        r = b * ntl + t
                        c_row = tmp.tile([128, CDIM], F32, tag="c_row")
                        nc.sync.dma_start(
                            out=c_row, in_=c_d[r * 128 : (r + 1) * 128, :]
                        )
                        for ct in range(CT):
                            pst = ps_mm.tile([128, 128], F32, tag="mm")
                            nc.tensor.transpose(
                                pst[:], c_row[:, ct * 128 : (ct + 1) * 128], ident
                            )
                            ts_ = slice(t * 128, (t + 1) * 128)
                            nc.vector.tensor_copy(cT32[:, ct, ts_], pst[:])
                            nc.scalar.copy(cT_bf[b][:, ct, ts_], pst[:])

                    h0 = act.tile([128, HT, nb], F32, tag="a0")
                    for ht in range(HT):
                        ps = ps_mm.tile([128, nb], F32, tag="mm")
                        for ct in range(CT):
                            nc.tensor.matmul(
                                ps[:],
                                pw0_s[:, ct, ht * 128 : (ht + 1) * 128],
                                cT32[:, ct, :],
                                start=(ct == 0),
                                stop=(ct == CT - 1),
                            )
                        nc.scalar.activation(
                            h0[:, ht, :], ps[:], AF.Relu,
                            bias=pbb[:, 0, ht : ht + 1],
                        )
                    h1 = act.tile([128, HT, nb], F32, tag="a1")
                    for h2 in range(HT):
                        ps = ps_mm.tile([128, nb], F32, tag="mm")
                        for h_1 in range(HT):
                            nc.tensor.matmul(
                                ps[:],
                                pw1_s[:, h_1, h2 * 128 : (h2 + 1) * 128],
                                h0[:, h_1, :],
                                start=(h_1 == 0),
                                stop=(h_1 == HT - 1),
                            )
                        nc.scalar.activation(
                            h1[:, h2, :], ps[:], AF.Relu,
                            bias=pbb[:, 1, h2 : h2 + 1],
                        )
                    for t in range(ntl):
                        r = b * ntl + t
                        psl = ps_mm.tile([128, K], F32, tag="mm")
                        for ht in range(HT):
                            nc.tensor.matmul(
                                psl[:],
                                h1[:, ht, t * 128 : (t + 1) * 128],
                                pw2_s[:, ht, :],
                                start=(ht == 0),
                                stop=(ht == HT - 1),
                            )
                        nc.vector.tensor_add(logits[:, r, :], psl[:], pb2_b)

            # ---- gumbel-max + softmax weights (vector; overlaps expert PE) ----
            # lg1 = log(u + EPS); then lg1 <- log(-lg1 + EPS) = -gumbel
            nc.scalar.activation(lg1, gu, AF.Ln, bias=eps_b)
            nc.scalar.activation(lg1, lg1, AF.Ln, bias=eps_b, scale=-1.0)
            # sc = logits + gumbel
            nc.vector.tensor_sub(sc, logits, lg1)
            with ExitStack() as ectx:
                tmp2 = ectx.enter_context(tc.tile_pool(name="tmp2", bufs=2))
                nzp = ectx.enter_context(tc.tile_pool(name="nz", bufs=3))
                act2 = ectx.enter_context(tc.tile_pool(name="act2", bufs=1))
                accp = ectx.enter_context(tc.tile_pool(name="accp", bufs=1))

                acc = accp.tile([128, rt, FDIM], F32, tag="acc")

                for r in range(rt):
                    # packed per-r temps: m1|mx|nmx|sm|rs | oh5 | ex | ps_t
                    tg = tmp2.tile([128, 20], F32, tag="tg")
                    m1 = tg[:, 0:1]
                    mx = tg[:, 1:2]
                    nmx = tg[:, 2:3]
                    sm = tg[:, 3:4]
                    rs = tg[:, 4:5]
                    oh5 = tg[:, 5:10]
                    ex = tg[:, 10:15]
                    ps_t = tg[:, 15:20]
                    nc.vector.tensor_reduce(m1, sc[:, r, :], axis=AX.X, op=ALU.max)
                    nc.vector.tensor_scalar(
                        oh5, sc[:, r, :], m1, WEIGHT, ALU.is_ge, ALU.mult
                    )
                    nc.vector.tensor_reduce(
                        mx, logits[:, r, :], axis=AX.X, op=ALU.max
                    )
                    nc.vector.tensor_scalar_mul(nmx, mx, -1.0)
                    nc.scalar.activation(ex, logits[:, r, :], AF.Exp, bias=nmx)
                    nc.vector.tensor_reduce(sm, ex, axis=AX.X, op=ALU.add)
                    nc.vector.reciprocal(rs, sm)
                    nc.vector.tensor_scalar_mul(ps_t, ex, rs)
                    nc.vector.tensor_add(ps_t, ps_t, oh5)
                    nc.vector.tensor_scalar_mul(
                        wgt[:, r, :], ps_t, 1.0 / (1.0 + WEIGHT)
                    )

                # ---- experts (bf16) ----
                for k in range(K):
                    gw0_s, gw1_s, gw2_s, gbb, bb = ew_next
                    if k + 1 < K:
                        ew_next = load_expert(k + 1)

                    for b in range(nbc):
                        g0 = act2.tile([128, HT, nb], BF16, tag="a0")
                        for ht in range(HT):
                            ps = ps_mm.tile([128, nb], F32, tag="mm")
                            for ct in range(CT):
                                nc.tensor.matmul(
                                    ps[:],
                                    gw0_s[:, ct, ht * 128 : (ht + 1) * 128],
                                    cT_bf[b][:, ct, :],
                                    start=(ct == 0),
                                    stop=(ct == CT - 1),
                                )
                            nc.scalar.activation(
                                g0[:, ht, :], ps[:], AF.Relu,
                                bias=gbb[:, 0, ht : ht + 1],
                            )
                        g1 = act2.tile([128, HT, nb], BF16, tag="a1")
                        for h2 in range(HT):
                            ps = ps_mm.tile([128, nb], F32, tag="mm")
                            for h_1 in range(HT):
                                nc.tensor.matmul(
                                    ps[:],
                                    gw1_s[:, h_1, h2 * 128 : (h2 + 1) * 128],
                                    g0[:, h_1, :],
                                    start=(h_1 == 0),
                                    stop=(h_1 == HT - 1),
                                )
                            nc.scalar.activation(
                                g1[:, h2, :], ps[:], AF.Relu,
                                bias=gbb[:, 1, h2 : h2 + 1],
                            )
                        # layer 3: row-major output [n, 2F]
                        for t in range(ntl):
                            r = b * ntl + t
                            ts_ = slice(t * 128, (t + 1) * 128)
                            ps_m = ps_l3.tile([128, FDIM], F32, tag="m")
                            ps_lv = ps_l3.tile([128, FDIM], F32, tag="lv")
                            for ht in range(HT):
                                nc.tensor.matmul(
                                    ps_m[:],
                                    g1[:, ht, ts_],
                                    gw2_s[:, ht, 0:FDIM],
                                    start=(ht == 0),
                                    stop=(ht == HT - 1),
                                )
                            for ht in range(HT):
                                nc.tensor.matmul(
                                    ps_lv[:],
                                    g1[:, ht, ts_],
                                    gw2_s[:, ht, FDIM:F2],
                                    start=(ht == 0),
                                    stop=(ht == HT - 1),
                                )
                            o_m = tmp2.tile([128, FDIM], F32, tag="o_m")
                            nc.vector.tensor_add(o_m[:], ps_m[:], bb[:, 0, :])
                            o_lv = tmp2.tile([128, FDIM], F32, tag="o_lv", bufs=1)
                            nc.vector.tensor_add(o_lv[:], ps_lv[:], bb[:, 1, :])
                            std = tmp2.tile([128, FDIM], F32, tag="std")
                            nc.scalar.activation(std[:], o_lv[:], AF.Exp, scale=0.5)
                            nz_t = nzp.tile([128, FDIM], F32, tag="nz")
                            nc.sync.dma_start(
                                out=nz_t,
                                in_=noise_d[k, r * 128 : (r + 1) * 128, :],
                            )
                            smp = tmp2.tile([128, FDIM], F32, tag="smp")
                            nc.vector.tensor_mul(smp[:], nz_t[:], std[:])
                            nc.vector.tensor_add(smp[:], smp[:], o_m[:])
                            wv = wgt[:, r, k : k + 1]
                            if k == 0:
                                nc.vector.tensor_scalar_mul(acc[:, r, :], smp[:], wv)
                            else:
                                nc.vector.tensor_scalar_mul(smp[:], smp[:], wv)
                                nc.vector.tensor_add(
                                    acc[:, r, :], acc[:, r, :], smp[:]
                                )
                                if k == K - 1:
                                    nc.sync.dma_start(
                                        out=out_d[r * 128 : (r + 1) * 128, :],
                                        in_=acc[:, r, :],
                                    )
    nc.compile()
    return nc


_PROGRAM_CACHE = {}


def get_program(nl: int):
    if nl not in _PROGRAM_CACHE:
        _PROGRAM_CACHE[nl] = build_program(nl)
    return _PROGRAM_CACHE[nl]


def make_in_maps(inputs: dict, n_cores: int = N_CORES):
    import ml_dtypes

    nl = inputs["c"].shape[0] // n_cores
    shared = {}
    for name in ("pw0", "pb0", "pw1", "pb1", "pw2", "pb2",
                 "gb0", "gb1", "gb2"):
        shared[name] = np.ascontiguousarray(
            np.asarray(inputs[name], dtype=np.float32)
        )
    for name in ("gw0", "gw1", "gw2"):
        shared[name] = np.ascontiguousarray(
            np.asarray(inputs[name], dtype=np.float32).astype(ml_dtypes.bfloat16)
        )
    c = np.asarray(inputs["c"], dtype=np.float32)
    noise = np.asarray(inputs["noise"], dtype=np.float32)
    gu = np.asarray(inputs["gumbel_u"], dtype=np.float32)
    in_maps = []
    for i in range(n_cores):
        rows = slice(i * nl, (i + 1) * nl)
        m = dict(shared)
        m["c"] = np.ascontiguousarray(c[rows])
        m["noise"] = np.ascontiguousarray(noise[:, rows, :])
        m["gumbel_u"] = np.ascontiguousarray(gu[rows])
        in_maps.append(m)
    return in_maps


def kernel(**inputs) -> np.ndarray:
    nc = get_program(N // N_CORES)
    in_maps = make_in_maps(inputs)
    res = run_bass_kernel_spmd(nc, in_maps, core_ids=list(range(N_CORES)))
    return np.concatenate(
        [res.results[i]["out"] for i in range(N_CORES)], axis=0
    )


# revision 8
# speedup vs baseline: 1.1045x; 1.0285x over previous
"""Trainium2 Bass kernel for nn_GaussianMixture (mixture-of-5-Gaussians sampler).

Strategy: data-parallel over the row dim N=16384 across 8 NeuronCores
(2048 rows/core), MLP weights replicated.

v2 vs baseline:
- Single 2048-row pass (no half-passes): the fp32 probs phase runs once,
  so the PE-clock throttle it triggers is paid once.
- Expert loop outermost: each expert's weights are DMA'd once per core
  (not once per half), double-buffered so expert k+1's weights stream in
  during expert k's compute.
- Expert matmuls in bf16 (weights converted host-side): ~5% faster issue
  rate than f32r and half the LDWEIGHTS/DMA bytes. Probs MLP stays fp32
  (the Gumbel argmax index is flip-sensitive to logit error).
- Activations stay feature-major ([h_features, n_rows]) through hidden
  layers; the final expert layer uses the hidden state as lhsT to emit
  row-major output, so sampling/weighted-combine run row-major.
"""
import sys

sys.path.insert(0, "/opt/trn_rl_repo")

from contextlib import ExitStack

import numpy as np

import concourse.bass as bass
import concourse.tile as tile
from concourse import bacc, mybir
from concourse.bass_utils import run_bass_kernel_spmd
from concourse.masks import make_identity

F32 = mybir.dt.float32
BF16 = mybir.dt.bfloat16
AF = mybir.ActivationFunctionType
ALU = mybir.AluOpType
AX = mybir.AxisListType

N_CORES = 8
N, CDIM, FDIM, HDIM, K = 16384, 512, 512, 1024, 5
F2 = 2 * FDIM
WEIGHT = 5.0
EPS = 1e-20

CT = CDIM // 128  # 4 c-feature tiles
HT = HDIM // 128  # 8 h-feature tiles


def build_program(nl: int):
    """Build the per-core program for nl rows (nl=2048 for the real run)."""
    nb = min(512, nl)     # n-block (matmul moving size)
    nbc = nl // nb        # n-blocks
    ntl = nb // 128       # row-tiles per n-block
    rt = nl // 128        # row-tiles total

    nc = bacc.Bacc("TRN2", target_bir_lowering=False, debug=False)

    c_d = nc.dram_tensor("c", [nl, CDIM], F32, kind="ExternalInput").ap()
    noise_d = nc.dram_tensor("noise", [K, nl, FDIM], F32, kind="ExternalInput").ap()
    gu_d = nc.dram_tensor("gumbel_u", [nl, K], F32, kind="ExternalInput").ap()
    pw0_d = nc.dram_tensor("pw0", [CDIM, HDIM], F32, kind="ExternalInput").ap()
    pb0_d = nc.dram_tensor("pb0", [HDIM], F32, kind="ExternalInput").ap()
    pw1_d = nc.dram_tensor("pw1", [HDIM, HDIM], F32, kind="ExternalInput").ap()
    pb1_d = nc.dram_tensor("pb1", [HDIM], F32, kind="ExternalInput").ap()
    pw2_d = nc.dram_tensor("pw2", [HDIM, K], F32, kind="ExternalInput").ap()
    pb2_d = nc.dram_tensor("pb2", [K], F32, kind="ExternalInput").ap()
    gw0_d = nc.dram_tensor("gw0", [K, CDIM, HDIM], BF16, kind="ExternalInput").ap()
    gb0_d = nc.dram_tensor("gb0", [K, HDIM], F32, kind="ExternalInput").ap()
    gw1_d = nc.dram_tensor("gw1", [K, HDIM, HDIM], BF16, kind="ExternalInput").ap()
    gb1_d = nc.dram_tensor("gb1", [K, HDIM], F32, kind="ExternalInput").ap()
    gw2_d = nc.dram_tensor("gw2", [K, HDIM, F2], BF16, kind="ExternalInput").ap()
    gb2_d = nc.dram_tensor("gb2", [K, F2], F32, kind="ExternalInput").ap()
    out_d = nc.dram_tensor("out", [nl, FDIM], F32, kind="ExternalOutput").ap()

    with tile.TileContext(nc) as tc:
        with ExitStack() as gctx:
            const = gctx.enter_context(tc.tile_pool(name="const", bufs=1))
            ps_mm = gctx.enter_context(
                tc.tile_pool(name="ps_mm", bufs=4, space="PSUM")
            )
            ps_l3 = gctx.enter_context(
                tc.tile_pool(name="ps_l3", bufs=2, space="PSUM")
            )
            sb = gctx.enter_context(tc.tile_pool(name="sb", bufs=1))
            ew = gctx.enter_context(tc.tile_pool(name="ew", bufs=2))

            # one packed const tile: identity | pb2 broadcast | eps
            constt = const.tile([128, 134], F32, tag="constt")
            ident = constt[:, 0:128]
            pb2_b = constt[:, 128:133]
            eps_b = constt[:, 133:134]
            make_identity(nc, ident)
            nc.gpsimd.dma_start(out=pb2_b, in_=pb2_d.partition_broadcast(128))
            nc.vector.memset(eps_b, EPS)

            # bf16 feature-major c for the expert MLPs, per block
            cT_bf = [
                sb.tile([128, CT, nb], BF16, tag=f"cTb{b}", name=f"cTb{b}")
                for b in range(nbc)
            ]
            # packed per-row small arrays: logits | gu | lg1 | sc | wgt
            smalls = sb.tile([128, 5, rt, K], F32, tag="smalls")
            logits = smalls[:, 0]
            gu = smalls[:, 1]
            lg1 = smalls[:, 2]
            sc = smalls[:, 3]
            wgt = smalls[:, 4]

            def load_expert(k):
                """Allocate + start DMA for expert k's weights (bf16) and
                biases. gw0/gw1/biases double-buffer; gw2 is single-buffered
                (only needed at L3) and issued LAST on the gpsimd queue so
                its WAR wait on expert k-1's L3 reads doesn't delay the
                bias transfers."""
                gw0_s = ew.tile([128, CT, HDIM], BF16, tag="gw0")
                nc.sync.dma_start(
                    out=gw0_s, in_=gw0_d[k].rearrange("(t p) h -> p t h", p=128)
                )
                gw1_s = ew.tile([128, HT, HDIM], BF16, tag="gw1")
                nc.sync.dma_start(
                    out=gw1_s, in_=gw1_d[k].rearrange("(t p) h -> p t h", p=128)
                )
                gbb = ew.tile([128, 2, HT], F32, tag="gbb")
                nc.gpsimd.dma_start(
                    out=gbb[:, 0, :], in_=gb0_d[k].rearrange("(t p) -> p t", p=128)
                )
                nc.gpsimd.dma_start(
                    out=gbb[:, 1, :], in_=gb1_d[k].rearrange("(t p) -> p t", p=128)
                )
                bb = ew.tile([128, 2, FDIM], F32, tag="bb")
                nc.gpsimd.dma_start(
                    out=bb[:, 0, :], in_=gb2_d[k, 0:FDIM].partition_broadcast(128)
                )
                nc.gpsimd.dma_start(
                    out=bb[:, 1, :], in_=gb2_d[k, FDIM:F2].partition_broadcast(128)
                )
                gw2_s = ew.tile([128, HT, F2], BF16, tag="gw2", bufs=1)
                nc.gpsimd.dma_start(
                    out=gw2_s, in_=gw2_d[k].rearrange("(t p) f -> p t f", p=128)
                )
                return gw0_s, gw1_s, gw2_s, gbb, bb

            # ---- probs MLP (fp32) ----
            # DMA queue order matters: c row-tiles go FIRST on the sync
            # queue (the transposes need them immediately); probs weights
            # go on the gpsimd queue (pw0 first — L0 needs it within a few
            # us); expert 0's big weights queue on sync BEHIND block 0's c
            # tiles (they aren't needed until the expert phase).
            ew_next = None
            with ExitStack() as pctx:
                pw = pctx.enter_context(tc.tile_pool(name="pw", bufs=1))
                act = pctx.enter_context(tc.tile_pool(name="act", bufs=1))
                tmp = pctx.enter_context(tc.tile_pool(name="tmp", bufs=4))

                pw0_s = pw.tile([128, CT, HDIM], F32, tag="w0")
                nc.gpsimd.dma_start(
                    out=pw0_s, in_=pw0_d.rearrange("(t p) h -> p t h", p=128)
                )
                pw1_s = pw.tile([128, HT, HDIM], F32, tag="w1")
                nc.gpsimd.dma_start(
                    out=pw1_s, in_=pw1_d.rearrange("(t p) h -> p t h", p=128)
                )
                pw2_s = pw.tile([128, HT, K], F32, tag="w2")
                nc.gpsimd.dma_start(
                    out=pw2_s, in_=pw2_d.rearrange("(t p) k -> p t k", p=128)
                )
                pbb = pw.tile([128, 2, HT], F32, tag="pbb")
                nc.gpsimd.dma_start(
                    out=pbb[:, 0, :], in_=pb0_d.rearrange("(t p) -> p t", p=128)
                )
                nc.gpsimd.dma_start(
                    out=pbb[:, 1, :], in_=pb1_d.rearrange("(t p) -> p t", p=128)
                )
                nc.gpsimd.dma_start(
                    out=gu, in_=gu_d.rearrange("(t p) k -> p t k", p=128)
                )

                for b in range(nbc):
                    # fp32 feature-major c for this block (rotating buffer)
                    cT32 = pw.tile([128, CT, nb], F32, tag="cT", bufs=2)
                    for t in range(ntl):
                        r = b * ntl + t
                        c_row = tmp.tile([128, CDIM], F32, tag="c_row")
                        nc.sync.dma_start(
                            out=c_row, in_=c_d[r * 128 : (r + 1) * 128, :]
                        )
                        for ct in range(CT):
                            pst = ps_mm.tile([128, 128], F32, tag="mm")
                            nc.tensor.transpose(
                                pst[:], c_row[:, ct * 128 : (ct + 1) * 128], ident
                            )
                            ts_ = slice(t * 128, (t + 1) * 128)
                            nc.vector.tensor_copy(cT32[:, ct, ts_], pst[:])
                            nc.scalar.copy(cT_bf[b][:, ct, ts_], pst[:])
                    if b == 0:
                        # expert 0 weight prefetch, behind block 0's c tiles
                        ew_next = load_expert(0)

                    h0 = act.tile([128, HT, nb], F32, tag="a0")
                    for ht in range(HT):
                        ps = ps_mm.tile([128, nb], F32, tag="mm")
                        for ct in range(CT):
                            nc.tensor.matmul(
                                ps[:],
                                pw0_s[:, ct, ht * 128 : (ht + 1) * 128],
                                cT32[:, ct, :],
                                start=(ct == 0),
                                stop=(ct == CT - 1),
                            )
                        nc.scalar.activation(
                            h0[:, ht, :], ps[:], AF.Relu,
                            bias=pbb[:, 0, ht : ht + 1],
                        )
                    h1 = act.tile([128, HT, nb], F32, tag="a1")
                    for h2 in range(HT):
                        ps = ps_mm.tile([128, nb], F32, tag="mm")
                        for h_1 in range(HT):
                            nc.tensor.matmul(
                                ps[:],
                                pw1_s[:, h_1, h2 * 128 : (h2 + 1) * 128],
                                h0[:, h_1, :],
                                start=(h_1 == 0),
                                stop=(h_1 == HT - 1),
                            )
                        nc.scalar.activation(
                            h1[:, h2, :], ps[:], AF.Relu,
                            bias=pbb[:, 1, h2 : h2 + 1],
                        )
                    for t in range(ntl):
                        r = b * ntl + t
                        psl = ps_mm.tile([128, K], F32, tag="mm")
                        for ht in range(HT):
                            nc.tensor.matmul(
                                psl[:],
                                h1[:, ht, t * 128 : (t + 1) * 128],
                                pw2_s[:, ht, :],
                                start=(ht == 0),
                                stop=(ht == HT - 1),
                            )
                        nc.vector.tensor_add(logits[:, r, :], psl[:], pb2_b)

            # ---- gumbel-max + softmax weights (vector; overlaps expert PE) ----
            # lg1 = log(u + EPS); then lg1 <- log(-lg1 + EPS) = -gumbel
            nc.scalar.activation(lg1, gu, AF.Ln, bias=eps_b)
            nc.scalar.activation(lg1, lg1, AF.Ln, bias=eps_b, scale=-1.0)
            # sc = logits + gumbel
            nc.vector.tensor_sub(sc, logits, lg1)
            with ExitStack() as ectx:
                tmp2 = ectx.enter_context(tc.tile_pool(name="tmp2", bufs=2))
                nzp = ectx.enter_context(tc.tile_pool(name="nz", bufs=3))
                act2 = ectx.enter_context(tc.tile_pool(name="act2", bufs=1))
                accp = ectx.enter_context(tc.tile_pool(name="accp", bufs=1))

                acc = accp.tile([128, rt, FDIM], F32, tag="acc")

                for r in range(rt):
                    # packed per-r temps: m1|mx|nmx|sm|rs | oh5 | ex | ps_t
                    tg = tmp2.tile([128, 20], F32, tag="tg")
                    m1 = tg[:, 0:1]
                    mx = tg[:, 1:2]
                    nmx = tg[:, 2:3]
                    sm = tg[:, 3:4]
                    rs = tg[:, 4:5]
                    oh5 = tg[:, 5:10]
                    ex = tg[:, 10:15]
                    ps_t = tg[:, 15:20]
                    nc.vector.tensor_reduce(m1, sc[:, r, :], axis=AX.X, op=ALU.max)
                    nc.vector.tensor_scalar(
                        oh5, sc[:, r, :], m1, WEIGHT, ALU.is_ge, ALU.mult
                    )
                    nc.vector.tensor_reduce(
                        mx, logits[:, r, :], axis=AX.X, op=ALU.max
                    )
                    nc.vector.tensor_scalar_mul(nmx, mx, -1.0)
                    nc.scalar.activation(ex, logits[:, r, :], AF.Exp, bias=nmx)
                    nc.vector.tensor_reduce(sm, ex, axis=AX.X, op=ALU.add)
                    nc.vector.reciprocal(rs, sm)
                    nc.vector.tensor_scalar_mul(ps_t, ex, rs)
                    nc.vector.tensor_add(ps_t, ps_t, oh5)
                    nc.vector.tensor_scalar_mul(
                        wgt[:, r, :], ps_t, 1.0 / (1.0 + WEIGHT)
                    )

                # ---- experts (bf16) ----
                for k in range(K):
                    gw0_s, gw1_s, gw2_s, gbb, bb = ew_next
                    if k + 1 < K:
                        ew_next = load_expert(k + 1)

                    for b in range(nbc):
                        g0 = act2.tile([128, HT, nb], BF16, tag="a0")
                        for ht in range(HT):
                            ps = ps_mm.tile([128, nb], F32, tag="mm")
                            for ct in range(CT):
                                nc.tensor.matmul(
                                    ps[:],
                                    gw0_s[:, ct, ht * 128 : (ht + 1) * 128],
                                    cT_bf[b][:, ct, :],
                                    start=(ct == 0),
                                    stop=(ct == CT - 1),
                                )
                            nc.scalar.activation(
                                g0[:, ht, :], ps[:], AF.Relu,
                                bias=gbb[:, 0, ht : ht + 1],
                            )
                        g1 = act2.tile([128, HT, nb], BF16, tag="a1")
                        for h2 in range(HT):
                            ps = ps_mm.tile([128, nb], F32, tag="mm")
                            for h_1 in range(HT):
                                nc.tensor.matmul(
                                    ps[:],
                                    gw1_s[:, h_1, h2 * 128 : (h2 + 1) * 128],
                                    g0[:, h_1, :],
                                    start=(h_1 == 0),
                                    stop=(h_1 == HT - 1),
                                )
                            nc.scalar.activation(
                                g1[:, h2, :], ps[:], AF.Relu,
                                bias=gbb[:, 1, h2 : h2 + 1],
                            )
                        # layer 3: row-major output [n, 2F]
                        for t in range(ntl):
                            r = b * ntl + t
                            ts_ = slice(t * 128, (t + 1) * 128)
                            ps_m = ps_l3.tile([128, FDIM], F32, tag="m")
                            ps_lv = ps_l3.tile([128, FDIM], F32, tag="lv")
                            for ht in range(HT):
                                nc.tensor.matmul(
                                    ps_m[:],
                                    g1[:, ht, ts_],
                                    gw2_s[:, ht, 0:FDIM],
                                    start=(ht == 0),
                                    stop=(ht == HT - 1),
                                )
                            for ht in range(HT):
                                nc.tensor.matmul(
                                    ps_lv[:],
                                    g1[:, ht, ts_],
                                    gw2_s[:, ht, FDIM:F2],
                                    start=(ht == 0),
                                    stop=(ht == HT - 1),
                                )
                            o_m = tmp2.tile([128, FDIM], F32, tag="o_m")
                            nc.vector.tensor_add(o_m[:], ps_m[:], bb[:, 0, :])
                            o_lv = tmp2.tile([128, FDIM], F32, tag="o_lv", bufs=1)
                            nc.vector.tensor_add(o_lv[:], ps_lv[:], bb[:, 1, :])
                            std = tmp2.tile([128, FDIM], F32, tag="std")
                            nc.scalar.activation(std[:], o_lv[:], AF.Exp, scale=0.5)
                            nz_t = nzp.tile([128, FDIM], F32, tag="nz")
                            nc.sync.dma_start(
                                out=nz_t,
                                in_=noise_d[k, r * 128 : (r + 1) * 128, :],
                            )
                            smp = tmp2.tile([128, FDIM], F32, tag="smp")
                            nc.vector.tensor_mul(smp[:], nz_t[:], std[:])
                            nc.vector.tensor_add(smp[:], smp[:], o_m[:])
                            wv = wgt[:, r, k : k + 1]
                            if k == 0:
                                nc.vector.tensor_scalar_mul(acc[:, r, :], smp[:], wv)
                            else:
                                nc.vector.tensor_scalar_mul(smp[:], smp[:], wv)
                                nc.vector.tensor_add(
                                    acc[:, r, :], acc[:, r, :], smp[:]
                                )
                                if k == K - 1:
                                    nc.sync.dma_start(
                                        out=out_d[r * 128 : (r + 1) * 128, :],
                                        in_=acc[:, r, :],
                                    )
    nc.compile()
    return nc


_PROGRAM_CACHE = {}


def get_program(nl: int):
    if nl not in _PROGRAM_CACHE:
        _PROGRAM_CACHE[nl] = build_program(nl)
    return _PROGRAM_CACHE[nl]


def make_in_maps(inputs: dict, n_cores: int = N_CORES):
    import ml_dtypes

    nl = inputs["c"].shape[0] // n_cores
    shared = {}
    for name in ("pw0", "pb0", "pw1", "pb1", "pw2", "pb2",
                 "gb0", "gb1", "gb2"):
        shared[name] = np.ascontiguousarray(
            np.asarray(inputs[name], dtype=np.float32)
        )
    for name in ("gw0", "gw1", "gw2"):
        shared[name] = np.ascontiguousarray(
            np.asarray(inputs[name], dtype=np.float32).astype(ml_dtypes.bfloat16)
        )
    c = np.asarray(inputs["c"], dtype=np.float32)
    noise = np.asarray(inputs["noise"], dtype=np.float32)
    gu = np.asarray(inputs["gumbel_u"], dtype=np.float32)
    in_maps = []
    for i in range(n_cores):
        rows = slice(i * nl, (i + 1) * nl)
        m = dict(shared)
        m["c"] = np.ascontiguousarray(c[rows])
        m["noise"] = np.ascontiguousarray(noise[:, rows, :])
        m["gumbel_u"] = np.ascontiguousarray(gu[rows])
        in_maps.append(m)
    return in_maps


def kernel(**inputs) -> np.ndarray:
    nc = get_program(N // N_CORES)
    in_maps = make_in_maps(inputs)
    res = run_bass_kernel_spmd(nc, in_maps, core_ids=list(range(N_CORES)))
    return np.concatenate(
        [res.results[i]["out"] for i in range(N_CORES)], axis=0
    )
